# revision 28
# baseline (speedup 1.0000x reference)
"""Trainium2 Bass kernel for GQA attention with RoPE (nn_Attention).

Reference (B=2, TQ=TKV=2048, D=2048, HQ=16, HKV=4, HD=128):
    q = Xq @ Wq; k = Xkv @ Wk; v = Xkv @ Wv
    q, k = rope(q, q_pos), rope(k, kv_pos)
    out = (causal_softmax(q k^T / sqrt(HD)) v) @ Wo   (kv head h//4 serves q head h)

Sharding: 8 cores = 2 batches x 4 query shards. Each core owns 8 interleaved
64-row query chunks (chunk i of core j is 4i + (j if i even else 3-j), which
balances the causal work exactly) and all 16 heads for those rows, so the
output projection needs no inter-core reduction. K/V projections are sharded
over the sequence (512 rows per core) and exchanged with AllGathers within
each batch group of 4 cores.

v3 structure:
  - V projection runs first (one 4-bank quad, kt-major so input DMA chunks
    pipeline), then K head-by-head with per-head rope, so the K/V exchange
    can be split into 4 per-kv-head AllGather pieces that fire early and
    pipeline: attention for kv head h only waits for piece h.
  - Emission interleaves Q-projection head groups with attention kv-head
    groups (one group of lookahead) so the PE always has ready work while
    pieces are in flight; Wo prefetch is spread between attention groups.
  - Attention processes each kv head in two passes of 2 query heads. Each
    pass's score tile is [128, 2*512] (2 PSUM banks) from a shared pool
    with bufs=2, and the ctx accumulator is [128, 2*512] with bufs=2 -
    8 banks total, giving真 double buffering: scores for block g+1 run
    while exp(g) drains, so the PE/ACT/DVE stages pipeline across g.
  - exp activations cover 2 heads per instruction (ACT costs ~352 cycles
    fixed per op); mask/accumulate ops use wide strided APs on DVE.

Scores are computed transposed (S^T[kv, q]) so attention*V needs no
transposes. The SPMD NEFF is identical on all cores, so the causal block
schedule is the conservative core-independent one: kv block g (128 rows)
runs against query columns [64*(g//2) : 512]; only the first 64-col
sub-block's validity differs per core and is handled by a multiplicative
0/1 mask shipped as data. Softmax denominators accumulate on DVE in f32r;
normalization is folded into a PSUM->SBUF multiply of the context.
"""
import numpy as np
import ml_dtypes

B = 2
T = 2048
D = 2048
HQ = 16
HKV = 4
HD = 128
HALF = HD // 2
N_CORES = 8
QROWS = 512
KVSH = 512
SCALE = 1.0 / float(np.sqrt(HD))
MAX_TIMESCALE = 10000.0

# 8 chunks of 64 query rows per core; chunk i lives in [4i, 4i+3]
CHUNKS = [[4 * i + (j if i % 2 == 0 else 3 - j) for i in range(8)]
          for j in range(4)]

_CACHE = {}


def _build():
    import concourse.mybir as mybir
    import concourse.tile as tile
    from concourse import bacc

    bf = mybir.dt.bfloat16
    f32 = mybir.dt.float32
    f32r = mybir.dt.float32r

    nc = bacc.Bacc("TRN2", target_bir_lowering=False, debug=False,
                   num_devices=N_CORES)

    # activations/weights arrive pre-arranged so SBUF loads are contiguous:
    # [128 partition, 16 k-tiles * cols]
    xqT = nc.dram_tensor("xqT", [128, 16 * QROWS], bf, kind="ExternalInput").ap()
    xkvT = nc.dram_tensor("xkvT", [128, 16 * KVSH], bf, kind="ExternalInput").ap()
    wq = nc.dram_tensor("wq", [128, HQ * 16 * HD], bf, kind="ExternalInput").ap()
    wk = nc.dram_tensor("wk", [128, 16 * HKV * HD], bf, kind="ExternalInput").ap()
    wv = nc.dram_tensor("wv", [128, 16 * HKV * HD], bf, kind="ExternalInput").ap()
    wo = nc.dram_tensor("wo", [HQ * HD, D], bf, kind="ExternalInput").ap()
    # cos/sin shipped pre-tiled: q 2-wide (head pairs), kv 1-wide
    cosq2 = nc.dram_tensor("cosq2", [HD, 2 * QROWS], bf, kind="ExternalInput").ap()
    sinq2 = nc.dram_tensor("sinq2", [HD, 2 * QROWS], bf, kind="ExternalInput").ap()
    coskv4 = nc.dram_tensor("coskv4", [HD, 4 * KVSH], bf, kind="ExternalInput").ap()
    sinkv4 = nc.dram_tensor("sinkv4", [HD, 4 * KVSH], bf, kind="ExternalInput").ap()
    dmask = nc.dram_tensor("dmask", [16, 128, 256], bf, kind="ExternalInput").ap()
    selbc = nc.dram_tensor("selbc", [4, 4 * HD], f32, kind="ExternalInput").ap()
    out = nc.dram_tensor("out", [QROWS, D], f32, kind="ExternalOutput").ap()

    Exp = mybir.ActivationFunctionType.Exp
    PIECE = 2 * 65536  # K^T head [128,512] ++ V head [512,128], bf16 elems

    with tile.TileContext(nc) as tc:
        with tc.tile_pool(name="dram", bufs=1, space="DRAM") as dram, \
             tc.tile_pool(name="persist", bufs=1) as persist:

            # ---------------- persistent SBUF tiles ----------------
            # roped Q^T per group of 4 heads: [hd, 4*512]
            qt_sb = [persist.tile([HD, 4 * QROWS], bf, name=f"qtg{g}")
                     for g in range(4)]
            kt_sb = [persist.tile([HD, T], bf, name=f"ktg{h}") for h in range(HKV)]
            v_sb = [persist.tile([128, 16 * HD], bf, name=f"vg{h}") for h in range(HKV)]
            # normalized context per head pair: [hd, 2*512]
            ctxn_sb = [persist.tile([HD, 2 * QROWS], bf, name=f"ctxn{p}")
                       for p in range(HQ // 2)]
            mask_sb = persist.tile([128, 16 * 128], bf, name="mask_sb")
            cq = persist.tile([HD, 2 * QROWS], bf, name="cq")
            sq = persist.tile([HD, 2 * QROWS], bf, name="sq")
            xq_sb = persist.tile([128, 16 * QROWS], bf, name="xq_sb")
            # pair-sums lhsT: sel2r[q] = [128, 2] f32r, only column q ones
            sel4_f = [persist.tile([128, 4], f32, name=f"sel4f_{q}") for q in range(4)]
            sel4r = [persist.tile([128, 4], f32r, name=f"sel4r_{q}") for q in range(4)]
            # bcast lhsT rows: sel128b[0:2, q*HD:] = ones at row q (q<2)
            sel128_f = persist.tile([4, 4 * HD], f32, name="sel128_f")
            sel128b = persist.tile([4, 4 * HD], bf, name="sel128b")


            # bounce buffers: piece h = K^T head h [128,512] ++ V head h [512,128]
            kv_in = dram.tile([HKV * PIECE], bf, name="kv_in")
            kv_out = dram.tile([4 * HKV * PIECE], bf, name="kv_out")

            # phase-B inputs phase A never touches: load from t=0 on the
            # gpsimd (SWDGE) queue, in disjoint SBUF, so Q-projection can
            # start right after the K projection with no WAR stall
            nc.gpsimd.dma_start(cq[:], cosq2)
            nc.gpsimd.dma_start(sq[:], sinq2)
            for ch in range(4):
                nc.gpsimd.dma_start(
                    xq_sb[:, ch * 4 * QROWS:(ch + 1) * 4 * QROWS],
                    xqT[:, ch * 4 * QROWS:(ch + 1) * 4 * QROWS])
            nc.gpsimd.dma_start(mask_sb.rearrange("p (g c) -> p g c", g=16),
                                dmask.rearrange("g p c -> p g c")[:, :, 0:128])

            # ---------------- phase A: K/V projections + AG pieces ----------
            psV = tc.tile_pool(name="psV", bufs=1, space="PSUM")
            psK = tc.tile_pool(name="psK", bufs=2, space="PSUM")
            sbA = tc.tile_pool(name="sbA", bufs=1)
            sbK = tc.tile_pool(name="sbK", bufs=2)
            psV_p = psV.__enter__()
            psK_p = psK.__enter__()
            sbA_p = sbA.__enter__()
            sbK_p = sbK.__enter__()

            wv_sb = sbA_p.tile([128, 16 * HKV * HD], bf, name="wv_sb")
            xkv_sb = sbA_p.tile([128, 16 * KVSH], bf, name="xkv_sb")
            for ch in range(8):
                nc.sync.dma_start(wv_sb[:, ch * 1024:(ch + 1) * 1024],
                                  wv[:, ch * 1024:(ch + 1) * 1024])
                nc.sync.dma_start(xkv_sb[:, ch * 2 * KVSH:(ch + 1) * 2 * KVSH],
                                  xkvT[:, ch * 2 * KVSH:(ch + 1) * 2 * KVSH])
            wk_sb = sbA_p.tile([128, 16 * HKV * HD], bf, name="wk_sb")
            nc.sync.dma_start(wk_sb[:], wk)
            ckv = sbA_p.tile([HD, KVSH], bf, name="ckv")
            skv = sbA_p.tile([HD, KVSH], bf, name="skv")
            nc.sync.dma_start(ckv[:], coskv4[:, 0:KVSH])
            nc.sync.dma_start(skv[:], sinkv4[:, 0:KVSH])

            # V quad [128 kv-in-block, (b, h, hd)]; kt-major to pipeline DMA
            vq = psV_p.tile([128, 4 * 512], f32, tag="vq", name="vq")
            for kt in range(16):
                for b in range(4):
                    nc.tensor.matmul(
                        vq[:, b * 512:(b + 1) * 512],
                        xkv_sb[:, kt * KVSH + b * 128:kt * KVSH + (b + 1) * 128],
                        wv_sb[:, kt * 512:(kt + 1) * 512],
                        start=(kt == 0), stop=(kt == 15))
            vsh = sbA_p.tile([128, 4 * 512], bf, name="vsh")
            nc.scalar.copy(vsh[:], vq[:])

            # K^T per head + rope + bounce + AG piece
            for h in range(HKV):
                kp = psK_p.tile([HD, KVSH], f32, tag="kp", name=f"kp{h}")
                for kt in range(16):
                    nc.tensor.matmul(
                        kp[:],
                        wk_sb[:, kt * 512 + h * HD:kt * 512 + (h + 1) * HD],
                        xkv_sb[:, kt * KVSH:(kt + 1) * KVSH],
                        start=(kt == 0), stop=(kt == 15))
                kraw = sbK_p.tile([HD, KVSH], bf, tag="kraw", name=f"kraw{h}")
                nc.scalar.copy(kraw[:], kp[:])
                ktr = sbK_p.tile([HD, KVSH], bf, tag="ktr", name=f"ktr{h}")
                t1 = sbK_p.tile([HALF, KVSH], bf, tag="t1", name=f"kt1_{h}")
                t2 = sbK_p.tile([HALF, KVSH], bf, tag="t2", name=f"kt2_{h}")
                nc.vector.tensor_mul(t1[:], kraw[0:HALF, :], ckv[0:HALF, :])
                nc.vector.tensor_mul(t2[:], kraw[HALF:HD, :], skv[HALF:HD, :])
                nc.vector.tensor_sub(ktr[0:HALF, :], t1[:], t2[:])
                t3 = sbK_p.tile([HALF, KVSH], bf, tag="t1", name=f"kt3_{h}")
                t4 = sbK_p.tile([HALF, KVSH], bf, tag="t2", name=f"kt4_{h}")
                nc.vector.tensor_mul(t3[:], kraw[0:HALF, :], skv[0:HALF, :])
                nc.vector.tensor_mul(t4[:], kraw[HALF:HD, :], ckv[HALF:HD, :])
                nc.vector.tensor_add(ktr[HALF:HD, :], t3[:], t4[:])

                base = h * PIECE
                nc.scalar.dma_start(
                    kv_in[base:base + 65536].rearrange("(p c) -> p c", p=HD),
                    ktr[:])
                nc.scalar.dma_start(
                    kv_in[base + 65536:base + 2 * 65536].rearrange(
                        "(b p c) -> p b c", b=4, p=128),
                    vsh.rearrange("p (b c) -> p b c", b=4)[:, :, h * HD:(h + 1) * HD])
                nc.gpsimd.collective_compute(
                    "AllGather", mybir.AluOpType.bypass,
                    replica_groups=[[0, 1, 2, 3], [4, 5, 6, 7]],
                    ins=[kv_in[base:base + PIECE].opt()],
                    outs=[kv_out[4 * base:4 * base + 4 * PIECE].opt()])

            sbK.__exit__(None, None, None)
            sbA.__exit__(None, None, None)
            psK.__exit__(None, None, None)
            psV.__exit__(None, None, None)

            # ---------------- phase B: Q proj interleaved with attention ----
            rtmp = tc.tile_pool(name="rtmp", bufs=2)
            rtmp_p = rtmp.__enter__()
            # shared [128,1024] PSUM pool: Q-proj pairs, score pairs, sums, bc
            ps2 = tc.tile_pool(name="ps2", bufs=2, space="PSUM")
            ps2_p = ps2.__enter__()
            ctx_ps = tc.tile_pool(name="ctx_ps", bufs=1, space="PSUM")
            ctx_p = ctx_ps.__enter__()
            psQ = tc.tile_pool(name="psQ", bufs=1, space="PSUM")
            psQ_p = psQ.__enter__()
            epool = tc.tile_pool(name="epool", bufs=6)
            e_p = epool.__enter__()
            apool = tc.tile_pool(name="apool", bufs=2)
            a_p = apool.__enter__()
            npool = tc.tile_pool(name="npool", bufs=2)
            n_p = npool.__enter__()
            bpool = tc.tile_pool(name="bpool", bufs=1)
            b_p = bpool.__enter__()
            # wq and wo share one 16-slot pool: wq slots load upfront,
            # wo reuses slot h once qproj_head(h) has consumed wq[h]
            wo_pool = tc.tile_pool(name="wo_pool", bufs=16, side="right")
            wo_p = wo_pool.__enter__()
            wo_sb = [None] * HQ
            wq_sb = []

            nc.sync.dma_start(sel128_f[:], selbc)
            nc.vector.tensor_copy(sel128b[:], sel128_f[:])
            for q in range(4):
                nc.vector.memset(sel4_f[q][:], 0.0)
                nc.vector.memset(sel4_f[q][:, q:q + 1], 1.0)
                nc.vector.tensor_copy(sel4r[q][:], sel4_f[q][:])
            for h in range(HQ):
                t = wo_p.tile([128, 16 * HD], bf, tag="w", name=f"wqsb{h}")
                nc.sync.dma_start(t[:], wq[:, h * 2048:(h + 1) * 2048])
                wq_sb.append(t)

            def _qproj_finish(h, ps):
                grp, qh = divmod(h, 4)
                qraw = rtmp_p.tile([HD, QROWS], bf, tag="qraw", name=f"qraw{h}")
                nc.scalar.copy(qraw[:], ps[:])
                qt = qt_sb[grp][:, qh * QROWS:(qh + 1) * QROWS]
                t1 = rtmp_p.tile([HALF, QROWS], bf, tag="t1", name=f"qt1_{h}")
                t2 = rtmp_p.tile([HALF, QROWS], bf, tag="t2", name=f"qt2_{h}")
                nc.vector.tensor_mul(t1[:], qraw[0:HALF, :], cq[0:HALF, 0:QROWS])
                nc.vector.tensor_mul(t2[:], qraw[HALF:HD, :], sq[HALF:HD, 0:QROWS])
                nc.vector.tensor_sub(qt[0:HALF, :], t1[:], t2[:])
                t3 = rtmp_p.tile([HALF, QROWS], bf, tag="t1", name=f"qt3_{h}")
                t4 = rtmp_p.tile([HALF, QROWS], bf, tag="t2", name=f"qt4_{h}")
                nc.vector.tensor_mul(t3[:], qraw[0:HALF, :], sq[0:HALF, 0:QROWS])
                nc.vector.tensor_mul(t4[:], qraw[HALF:HD, :], cq[HALF:HD, 0:QROWS])
                nc.vector.tensor_add(qt[HALF:HD, :], t3[:], t4[:])

            def qproj_head(h):
                # one head -> one 512-col slice of qt_sb[h//4]
                ps = psQ_p.tile([HD, QROWS], f32, tag="qp", name=f"qps{h}")
                for kt in range(16):
                    nc.tensor.matmul(
                        ps[:],
                        wq_sb[h][:, kt * HD:(kt + 1) * HD],
                        xq_sb[:, kt * QROWS:(kt + 1) * QROWS],
                        start=(kt == 0), stop=(kt == 15))
                _qproj_finish(h, ps)

            # filler: qproj matmuls dribbled into attention-pass bubbles
            # (two per kv block, between the score and context matmuls)
            fillq = {"heads": [], "kt": 0, "ps": None}

            def fill_set(heads):
                fillq["heads"] = list(heads)

            def emit_fill(n):
                for _ in range(n):
                    if not fillq["heads"]:
                        return
                    h = fillq["heads"][0]
                    kt = fillq["kt"]
                    if kt == 0:
                        fillq["ps"] = psQ_p.tile([HD, QROWS], f32, tag="qp",
                                                 name=f"qps{h}")
                    nc.tensor.matmul(
                        fillq["ps"][:],
                        wq_sb[h][:, kt * HD:(kt + 1) * HD],
                        xq_sb[:, kt * QROWS:(kt + 1) * QROWS],
                        start=(kt == 0), stop=(kt == 15))
                    if kt == 15:
                        _qproj_finish(h, fillq["ps"])
                        fillq["heads"].pop(0)
                        fillq["kt"] = 0
                        fillq["ps"] = None
                    else:
                        fillq["kt"] = kt + 1

            def attn_unpack(kvh):
                base4 = 4 * kvh * PIECE
                for r in range(4):
                    rb = base4 + r * PIECE
                    nc.gpsimd.dma_start(
                        kt_sb[kvh][:, r * 512:(r + 1) * 512],
                        kv_out[rb:rb + 65536].rearrange("(p c) -> p c", p=HD))
                    nc.gpsimd.dma_start(
                        v_sb[kvh][:, r * 512:(r + 1) * 512].rearrange(
                            "p (b c) -> p b c", b=4),
                        kv_out[rb + 65536:rb + 2 * 65536].rearrange(
                            "(b p c) -> p b c", b=4, p=128))

            def attn_pass(kvh, pr):
                # one pass of 2 query heads over all kv blocks
                if True:
                    ctx = ctx_p.tile([HD, 2 * QROWS], f32, tag="ctx",
                                     name=f"ctx{kvh}_{pr}")
                    acc = a_p.tile([128, 2 * QROWS], f32r, tag="acc",
                                   name=f"acc{kvh}_{pr}")
                    for g in range(16):
                        off = 64 * (g // 2)
                        ng = QROWS - off
                        sc = ps2_p.tile([128, 2 * QROWS], f32, tag="p2",
                                        name=f"sc{kvh}_{g}_{pr}")
                        for hl in range(2):
                            qh = 2 * pr + hl
                            nc.tensor.matmul(
                                sc[:, hl * QROWS:hl * QROWS + ng],
                                kt_sb[kvh][:, g * 128:(g + 1) * 128],
                                qt_sb[kvh][:, qh * QROWS + off:(qh + 1) * QROWS],
                                start=True, stop=True)
                        emit_fill(2)
                        sc_src = sc.rearrange("p (q c) -> p q c", q=2)[:, :, 0:ng]
                        et = e_p.tile([128, 2 * QROWS], bf, tag="exp",
                                      name=f"et{kvh}_{g}_{pr}")
                        nc.scalar.activation(
                            et.rearrange("p (q c) -> p q c", q=2)[:, :, 0:ng],
                            sc_src,
                            Exp, scale=SCALE)
                        nc.vector.tensor_mul(
                            et.rearrange("p (q c) -> p q c", q=2)[:, :, 0:64],
                            et.rearrange("p (q c) -> p q c", q=2)[:, :, 0:64],
                            mask_sb.rearrange("p (g q c) -> p g q c", g=16, q=2)
                            [:, g, :, :])
                        with nc.allow_low_precision(reason="f32r softmax sums"):
                            if g == 0:
                                nc.vector.tensor_copy(acc[:], et[:])
                            else:
                                nc.vector.tensor_add(
                                    acc.rearrange("p (q c) -> p q c", q=2)
                                    [:, :, off:QROWS],
                                    acc.rearrange("p (q c) -> p q c", q=2)
                                    [:, :, off:QROWS],
                                    et.rearrange("p (q c) -> p q c", q=2)
                                    [:, :, 0:ng])
                        for hl in range(2):
                            nc.tensor.matmul(
                                ctx[:, hl * QROWS + off:(hl + 1) * QROWS],
                                v_sb[kvh][:, g * 128:(g + 1) * 128],
                                et[:, hl * QROWS:hl * QROWS + ng],
                                start=(g == 0), stop=(g == 15),
                                skip_group_check=True)
                    # tail: denominators -> reciprocal -> broadcast -> normalize
                    sums = ps2_p.tile([2, QROWS], f32, tag="p2",
                                      name=f"sums{kvh}_{pr}")
                    for hl in range(2):
                        nc.tensor.matmul(
                            sums[:], sel4r[hl][:, 0:2],
                            acc[:, hl * QROWS:(hl + 1) * QROWS],
                            start=(hl == 0), stop=(hl == 1),
                            skip_group_check=True)
                    recf = n_p.tile([2, QROWS], f32, tag="recf",
                                    name=f"recf{kvh}_{pr}")
                    nc.vector.reciprocal_approx_fast(recf[:], sums[:])
                    recb = n_p.tile([2, QROWS], bf, tag="recb",
                                    name=f"recb{kvh}_{pr}")
                    with nc.allow_low_precision(reason="bf16 softmax denominators"):
                        nc.vector.tensor_copy(recb[:], recf[:])
                    bps = ps2_p.tile([HD, 2 * QROWS], f32, tag="p2",
                                     name=f"bps{kvh}_{pr}")
                    for hl in range(2):
                        nc.tensor.matmul(
                            bps[:, hl * QROWS:(hl + 1) * QROWS],
                            sel128b[0:2, hl * HD:(hl + 1) * HD], recb[:],
                            start=True, stop=True)
                    bsb = b_p.tile([HD, 2 * QROWS], f32, tag="bsb",
                                   name=f"bsb{kvh}_{pr}")
                    nc.scalar.copy(bsb[:], bps[:])
                    nc.vector.tensor_mul(ctxn_sb[2 * kvh + pr][:], ctx[:], bsb[:])

            def wo_prefetch(kvh):
                for h in range(4 * kvh, 4 * kvh + 4):
                    t = wo_p.tile([HD, D], bf, tag="w", name=f"wosb{h}")
                    nc.sync.dma_start(t[:], wo[h * HD:(h + 1) * HD, :])
                    wo_sb[h] = t

            # 8 heads of lookahead before attention; heads 8-15 dribble
            # into the first four passes' bubbles two matmuls per kv block
            for h in range(8):
                qproj_head(h)
            attn_unpack(0)
            fill_set([8, 9])
            attn_pass(0, 0)
            fill_set([10, 11])
            attn_pass(0, 1)
            wo_prefetch(0)
            attn_unpack(1)
            fill_set([12, 13])
            attn_pass(1, 0)
            fill_set([14, 15])
            attn_pass(1, 1)
            wo_prefetch(1)
            attn_unpack(2)
            attn_pass(2, 0)
            attn_pass(2, 1)
            wo_prefetch(2)
            attn_unpack(3)
            attn_pass(3, 0)
            attn_pass(3, 1)
            wo_prefetch(3)

            for pool in [bpool, npool, apool, epool, psQ, ctx_ps, ps2,
                         rtmp]:
                pool.__exit__(None, None, None)

            # ---------------- phase C: output projection ----------------
            with tc.tile_pool(name="out_ps", bufs=2, space="PSUM") as out_ps, \
                 tc.tile_pool(name="osb_pool", bufs=2) as osb_pool:
                for c in range(4):
                    ps = out_ps.tile([128, D], f32, tag="ops", name=f"ops{c}")
                    for sl in range(4):
                        for p in range(HQ // 2):
                            for hl in range(2):
                                h = 2 * p + hl
                                nc.tensor.matmul(
                                    ps[:, sl * 512:(sl + 1) * 512],
                                    ctxn_sb[p][:, hl * QROWS + c * 128:
                                               hl * QROWS + (c + 1) * 128],
                                    wo_sb[h][:, sl * 512:(sl + 1) * 512],
                                    start=(h == 0), stop=(h == HQ - 1))
                    osb = osb_pool.tile([128, D], f32, tag="osb", name=f"osb{c}")
                    nc.scalar.copy(osb[:], ps[:])
                    nc.sync.dma_start(out[c * 128:(c + 1) * 128, :], osb[:])
            wo_pool.__exit__(None, None, None)

    nc.compile()
    return nc


def _prep_core_inputs(c, Xq, Xkv, wq2, wk2, wv2, wo2, q_positions, kv_positions):
    bfl = ml_dtypes.bfloat16
    b, j = divmod(c, 4)
    chunks = CHUNKS[j]
    qrows = np.concatenate([np.arange(64 * ch, 64 * ch + 64) for ch in chunks])
    kvrows = np.arange(512 * j, 512 * j + 512)

    inv_freq = 1.0 / (MAX_TIMESCALE **
                      (2.0 * np.arange(HALF, dtype=np.float32) / HD))
    pq = q_positions[b][qrows].astype(np.float32)
    pk = kv_positions[b][kvrows].astype(np.float32)
    fq = inv_freq[:, None] * pq[None, :]
    fk = inv_freq[:, None] * pk[None, :]

    # validity mask for the first 64-col sub-block of each kv block:
    # chunk i0 = g//2, columns are rows 64*c0..64*c0+63, valid iff kv <= q
    dm = np.zeros((16, 128, 64), dtype=np.float32)
    for g in range(16):
        c0 = chunks[g // 2]
        kv_idx = 128 * g + np.arange(128)[:, None]
        q_idx = 64 * c0 + np.arange(64)[None, :]
        dm[g] = (kv_idx <= q_idx).astype(np.float32)
    dm = np.tile(dm[:, :, None, :], (1, 1, 4, 1)).reshape(16, 128, 256)

    cosq = np.concatenate([np.cos(fq)] * 2, axis=0)   # [128, 512]
    sinq = np.concatenate([np.sin(fq)] * 2, axis=0)
    coskv = np.concatenate([np.cos(fk)] * 2, axis=0)
    sinkv = np.concatenate([np.sin(fk)] * 2, axis=0)

    xq_dev = np.ascontiguousarray(
        Xq[b][qrows, :].T.reshape(16, 128, QROWS).transpose(1, 0, 2)
        .reshape(128, 16 * QROWS))
    xkv_dev = np.ascontiguousarray(
        Xkv[b][kvrows, :].T.reshape(16, 128, KVSH).transpose(1, 0, 2)
        .reshape(128, 16 * KVSH))
    return dict(
        xqT=xq_dev.astype(bfl),
        xkvT=xkv_dev.astype(bfl),
        wq=wq2, wk=wk2, wv=wv2, wo=wo2,
        cosq2=np.tile(cosq, (1, 2)).astype(bfl),
        sinq2=np.tile(sinq, (1, 2)).astype(bfl),
        coskv4=np.tile(coskv, (1, 4)).astype(bfl),
        sinkv4=np.tile(sinkv, (1, 4)).astype(bfl),
        dmask=dm.astype(bfl),
        selbc=_selbc(),
    )


def _selbc():
    s = np.zeros((4, 4 * HD), dtype=np.float32)
    for q in range(4):
        s[q, q * HD:(q + 1) * HD] = 1.0
    return s


def kernel(Xq, Xkv, Wq, Wk, Wv, Wo, q_positions, kv_positions):
    from concourse import bass_utils

    Xq = np.asarray(Xq, dtype=np.float32)
    Xkv = np.asarray(Xkv, dtype=np.float32)
    Wq = np.asarray(Wq, dtype=np.float32)
    Wk = np.asarray(Wk, dtype=np.float32)
    Wv = np.asarray(Wv, dtype=np.float32)
    Wo = np.asarray(Wo, dtype=np.float32)
    q_positions = np.asarray(q_positions)
    kv_positions = np.asarray(kv_positions)

    if "nc" not in _CACHE:
        _CACHE["nc"] = _build()
    nc = _CACHE["nc"]

    bfl = ml_dtypes.bfloat16
    # wq: [128 p, h*16kt*128] so each head's lhsT block is contiguous
    wq2 = np.ascontiguousarray(
        Wq.reshape(16, 128, HQ, HD).transpose(1, 2, 0, 3)
        .reshape(128, HQ * 16 * HD)).astype(bfl)
    wk2 = np.ascontiguousarray(
        Wk.reshape(16, 128, HKV * HD).transpose(1, 0, 2)
        .reshape(128, 16 * HKV * HD)).astype(bfl)
    wv2 = np.ascontiguousarray(
        Wv.reshape(16, 128, HKV * HD).transpose(1, 0, 2)
        .reshape(128, 16 * HKV * HD)).astype(bfl)
    wo2 = np.ascontiguousarray(Wo.reshape(HQ * HD, D)).astype(bfl)

    in_maps = [_prep_core_inputs(c, Xq, Xkv, wq2, wk2, wv2, wo2,
                                 q_positions, kv_positions)
               for c in range(N_CORES)]

    res = bass_utils.run_bass_kernel_spmd(
        nc, in_maps, core_ids=list(range(N_CORES)),
        **_CACHE.get("run_kwargs", {}))
    _CACHE["last_results"] = res

    out = np.empty((B, T, D), dtype=np.float32)
    for c in range(N_CORES):
        b, j = divmod(c, 4)
        core_out = res.results[c]["out"]
        for i, ch in enumerate(CHUNKS[j]):
            out[b, 64 * ch:64 * ch + 64, :] = core_out[64 * i:64 * i + 64, :]
    return out



# revision 29
# speedup vs baseline: 1.0183x; 1.0183x over previous
"""Trainium2 Bass kernel for GQA attention with RoPE (nn_Attention).

Reference (B=2, TQ=TKV=2048, D=2048, HQ=16, HKV=4, HD=128):
    q = Xq @ Wq; k = Xkv @ Wk; v = Xkv @ Wv
    q, k = rope(q, q_pos), rope(k, kv_pos)
    out = (causal_softmax(q k^T / sqrt(HD)) v) @ Wo   (kv head h//4 serves q head h)

Sharding: 8 cores = 2 batches x 4 query shards. Each core owns 8 interleaved
64-row query chunks (chunk i of core j is 4i + (j if i even else 3-j), which
balances the causal work exactly) and all 16 heads for those rows, so the
output projection needs no inter-core reduction. K/V projections are sharded
over the sequence (512 rows per core) and exchanged with AllGathers within
each batch group of 4 cores.

v3 structure:
  - V projection runs first (one 4-bank quad, kt-major so input DMA chunks
    pipeline), then K head-by-head with per-head rope, so the K/V exchange
    can be split into 4 per-kv-head AllGather pieces that fire early and
    pipeline: attention for kv head h only waits for piece h.
  - Emission interleaves Q-projection head groups with attention kv-head
    groups (one group of lookahead) so the PE always has ready work while
    pieces are in flight; Wo prefetch is spread between attention groups.
  - Attention processes each kv head in two passes of 2 query heads. Each
    pass's score tile is [128, 2*512] (2 PSUM banks) from a shared pool
    with bufs=2, and the ctx accumulator is [128, 2*512] with bufs=2 -
    8 banks total, giving真 double buffering: scores for block g+1 run
    while exp(g) drains, so the PE/ACT/DVE stages pipeline across g.
  - exp activations cover 2 heads per instruction (ACT costs ~352 cycles
    fixed per op); mask/accumulate ops use wide strided APs on DVE.

Scores are computed transposed (S^T[kv, q]) so attention*V needs no
transposes. The SPMD NEFF is identical on all cores, so the causal block
schedule is the conservative core-independent one: kv block g (128 rows)
runs against query columns [64*(g//2) : 512]; only the first 64-col
sub-block's validity differs per core and is handled by a multiplicative
0/1 mask shipped as data. Softmax denominators accumulate on DVE in f32r;
normalization is folded into a PSUM->SBUF multiply of the context.
"""
import numpy as np
import ml_dtypes

B = 2
T = 2048
D = 2048
HQ = 16
HKV = 4
HD = 128
HALF = HD // 2
N_CORES = 8
QROWS = 512
KVSH = 512
SCALE = 1.0 / float(np.sqrt(HD))
MAX_TIMESCALE = 10000.0

# 8 chunks of 64 query rows per core; chunk i lives in [4i, 4i+3]
CHUNKS = [[4 * i + (j if i % 2 == 0 else 3 - j) for i in range(8)]
          for j in range(4)]

_CACHE = {}


def _build():
    import concourse.mybir as mybir
    import concourse.tile as tile
    from concourse import bacc

    bf = mybir.dt.bfloat16
    f32 = mybir.dt.float32
    f32r = mybir.dt.float32r

    nc = bacc.Bacc("TRN2", target_bir_lowering=False, debug=False,
                   num_devices=N_CORES)

    # activations/weights arrive pre-arranged so SBUF loads are contiguous:
    # [128 partition, 16 k-tiles * cols]
    xqT = nc.dram_tensor("xqT", [128, 16 * QROWS], bf, kind="ExternalInput").ap()
    xkvT = nc.dram_tensor("xkvT", [128, 16 * KVSH], bf, kind="ExternalInput").ap()
    wq = nc.dram_tensor("wq", [128, HQ * 16 * HD], bf, kind="ExternalInput").ap()
    wk = nc.dram_tensor("wk", [128, 16 * HKV * HD], bf, kind="ExternalInput").ap()
    wv = nc.dram_tensor("wv", [128, 16 * HKV * HD], bf, kind="ExternalInput").ap()
    wo = nc.dram_tensor("wo", [HQ * HD, D], bf, kind="ExternalInput").ap()
    # cos/sin shipped pre-tiled: q 2-wide (head pairs), kv 1-wide
    cosq2 = nc.dram_tensor("cosq2", [HD, 2 * QROWS], bf, kind="ExternalInput").ap()
    sinq2 = nc.dram_tensor("sinq2", [HD, 2 * QROWS], bf, kind="ExternalInput").ap()
    coskv4 = nc.dram_tensor("coskv4", [HD, 4 * KVSH], bf, kind="ExternalInput").ap()
    sinkv4 = nc.dram_tensor("sinkv4", [HD, 4 * KVSH], bf, kind="ExternalInput").ap()
    dmask = nc.dram_tensor("dmask", [16, 128, 256], bf, kind="ExternalInput").ap()
    selbc = nc.dram_tensor("selbc", [4, 4 * HD], f32, kind="ExternalInput").ap()
    out = nc.dram_tensor("out", [QROWS, D], f32, kind="ExternalOutput").ap()

    Exp = mybir.ActivationFunctionType.Exp
    PIECE = 2 * 65536  # K^T head [128,512] ++ V head [512,128], bf16 elems

    with tile.TileContext(nc) as tc:
        with tc.tile_pool(name="dram", bufs=1, space="DRAM") as dram, \
             tc.tile_pool(name="persist", bufs=1) as persist:

            # ---------------- persistent SBUF tiles ----------------
            # roped Q^T per group of 4 heads: [hd, 4*512]
            qt_sb = [persist.tile([HD, 4 * QROWS], bf, name=f"qtg{g}")
                     for g in range(4)]
            kt_sb = [persist.tile([HD, T], bf, name=f"ktg{h}") for h in range(HKV)]
            v_sb = [persist.tile([128, 16 * HD], bf, name=f"vg{h}") for h in range(HKV)]
            # normalized context per head pair: [hd, 2*512]
            ctxn_sb = [persist.tile([HD, 2 * QROWS], bf, name=f"ctxn{p}")
                       for p in range(HQ // 2)]
            mask_sb = persist.tile([128, 16 * 128], bf, name="mask_sb")
            cq = persist.tile([HD, 2 * QROWS], bf, name="cq")
            sq = persist.tile([HD, 2 * QROWS], bf, name="sq")
            # pair-sums lhsT: sel2r[q] = [128, 2] f32r, only column q ones
            sel4_f = [persist.tile([128, 4], f32, name=f"sel4f_{q}") for q in range(4)]
            sel4r = [persist.tile([128, 4], f32r, name=f"sel4r_{q}") for q in range(4)]
            # bcast lhsT rows: sel128b[0:2, q*HD:] = ones at row q (q<2)
            sel128_f = persist.tile([4, 4 * HD], f32, name="sel128_f")
            sel128b = persist.tile([4, 4 * HD], bf, name="sel128b")


            # bounce buffers: piece h = K^T head h [128,512] ++ V head h [512,128]
            kv_in = dram.tile([HKV * PIECE], bf, name="kv_in")
            kv_out = dram.tile([4 * HKV * PIECE], bf, name="kv_out")

            # ---------------- phase A: K/V projections + AG pieces ----------
            psV = tc.tile_pool(name="psV", bufs=1, space="PSUM")
            psK = tc.tile_pool(name="psK", bufs=2, space="PSUM")
            sbA = tc.tile_pool(name="sbA", bufs=1)
            sbK = tc.tile_pool(name="sbK", bufs=2)
            psV_p = psV.__enter__()
            psK_p = psK.__enter__()
            sbA_p = sbA.__enter__()
            sbK_p = sbK.__enter__()

            wv_sb = sbA_p.tile([128, 16 * HKV * HD], bf, name="wv_sb")
            xkv_sb = sbA_p.tile([128, 16 * KVSH], bf, name="xkv_sb")
            for ch in range(8):
                nc.sync.dma_start(wv_sb[:, ch * 1024:(ch + 1) * 1024],
                                  wv[:, ch * 1024:(ch + 1) * 1024])
                nc.sync.dma_start(xkv_sb[:, ch * 2 * KVSH:(ch + 1) * 2 * KVSH],
                                  xkvT[:, ch * 2 * KVSH:(ch + 1) * 2 * KVSH])
            wk_sb = sbA_p.tile([128, 16 * HKV * HD], bf, name="wk_sb")
            nc.sync.dma_start(wk_sb[:], wk)
            ckv = sbA_p.tile([HD, KVSH], bf, name="ckv")
            skv = sbA_p.tile([HD, KVSH], bf, name="skv")
            nc.sync.dma_start(ckv[:], coskv4[:, 0:KVSH])
            nc.sync.dma_start(skv[:], sinkv4[:, 0:KVSH])

            # V quad [128 kv-in-block, (b, h, hd)]; kt-major to pipeline DMA
            vq = psV_p.tile([128, 4 * 512], f32, tag="vq", name="vq")
            for kt in range(16):
                for b in range(4):
                    nc.tensor.matmul(
                        vq[:, b * 512:(b + 1) * 512],
                        xkv_sb[:, kt * KVSH + b * 128:kt * KVSH + (b + 1) * 128],
                        wv_sb[:, kt * 512:(kt + 1) * 512],
                        start=(kt == 0), stop=(kt == 15))
            vsh = sbA_p.tile([128, 4 * 512], bf, name="vsh")
            nc.scalar.copy(vsh[:], vq[:])

            # K^T per head + rope + bounce + AG piece
            for h in range(HKV):
                kp = psK_p.tile([HD, KVSH], f32, tag="kp", name=f"kp{h}")
                for kt in range(16):
                    nc.tensor.matmul(
                        kp[:],
                        wk_sb[:, kt * 512 + h * HD:kt * 512 + (h + 1) * HD],
                        xkv_sb[:, kt * KVSH:(kt + 1) * KVSH],
                        start=(kt == 0), stop=(kt == 15))
                kraw = sbK_p.tile([HD, KVSH], bf, tag="kraw", name=f"kraw{h}")
                nc.scalar.copy(kraw[:], kp[:])
                ktr = sbK_p.tile([HD, KVSH], bf, tag="ktr", name=f"ktr{h}")
                t1 = sbK_p.tile([HALF, KVSH], bf, tag="t1", name=f"kt1_{h}")
                t2 = sbK_p.tile([HALF, KVSH], bf, tag="t2", name=f"kt2_{h}")
                nc.vector.tensor_mul(t1[:], kraw[0:HALF, :], ckv[0:HALF, :])
                nc.vector.tensor_mul(t2[:], kraw[HALF:HD, :], skv[HALF:HD, :])
                nc.vector.tensor_sub(ktr[0:HALF, :], t1[:], t2[:])
                t3 = sbK_p.tile([HALF, KVSH], bf, tag="t1", name=f"kt3_{h}")
                t4 = sbK_p.tile([HALF, KVSH], bf, tag="t2", name=f"kt4_{h}")
                nc.vector.tensor_mul(t3[:], kraw[0:HALF, :], skv[0:HALF, :])
                nc.vector.tensor_mul(t4[:], kraw[HALF:HD, :], ckv[HALF:HD, :])
                nc.vector.tensor_add(ktr[HALF:HD, :], t3[:], t4[:])

                base = h * PIECE
                nc.scalar.dma_start(
                    kv_in[base:base + 65536].rearrange("(p c) -> p c", p=HD),
                    ktr[:])
                nc.scalar.dma_start(
                    kv_in[base + 65536:base + 2 * 65536].rearrange(
                        "(b p c) -> p b c", b=4, p=128),
                    vsh.rearrange("p (b c) -> p b c", b=4)[:, :, h * HD:(h + 1) * HD])
                nc.gpsimd.collective_compute(
                    "AllGather", mybir.AluOpType.bypass,
                    replica_groups=[[0, 1, 2, 3], [4, 5, 6, 7]],
                    ins=[kv_in[base:base + PIECE].opt()],
                    outs=[kv_out[4 * base:4 * base + 4 * PIECE].opt()])

            sbK.__exit__(None, None, None)
            sbA.__exit__(None, None, None)
            psK.__exit__(None, None, None)
            psV.__exit__(None, None, None)

            # ---------------- phase B: Q proj interleaved with attention ----
            sbQ = tc.tile_pool(name="sbQ", bufs=1)
            sbQ_p = sbQ.__enter__()
            rtmp = tc.tile_pool(name="rtmp", bufs=2)
            rtmp_p = rtmp.__enter__()
            # shared [128,1024] PSUM pool: Q-proj pairs, score pairs, sums, bc
            ps2 = tc.tile_pool(name="ps2", bufs=2, space="PSUM")
            ps2_p = ps2.__enter__()
            ctx_ps = tc.tile_pool(name="ctx_ps", bufs=1, space="PSUM")
            ctx_p = ctx_ps.__enter__()
            psQ = tc.tile_pool(name="psQ", bufs=1, space="PSUM")
            psQ_p = psQ.__enter__()
            epool = tc.tile_pool(name="epool", bufs=6)
            e_p = epool.__enter__()
            apool = tc.tile_pool(name="apool", bufs=2)
            a_p = apool.__enter__()
            npool = tc.tile_pool(name="npool", bufs=2)
            n_p = npool.__enter__()
            bpool = tc.tile_pool(name="bpool", bufs=1)
            b_p = bpool.__enter__()
            # wq and wo share one 16-slot pool: wq slots load upfront,
            # wo reuses slot h once qproj_head(h) has consumed wq[h]
            wo_pool = tc.tile_pool(name="wo_pool", bufs=16, side="right")
            wo_p = wo_pool.__enter__()
            wo_sb = [None] * HQ
            wq_sb = []

            xq_sb = sbQ_p.tile([128, 16 * QROWS], bf, name="xq_sb")
            for ch in range(8):
                nc.sync.dma_start(xq_sb[:, ch * 2 * QROWS:(ch + 1) * 2 * QROWS],
                                  xqT[:, ch * 2 * QROWS:(ch + 1) * 2 * QROWS])
            nc.sync.dma_start(cq[:], cosq2)
            nc.sync.dma_start(sq[:], sinq2)
            nc.sync.dma_start(mask_sb.rearrange("p (g c) -> p g c", g=16),
                              dmask.rearrange("g p c -> p g c")[:, :, 0:128])
            nc.sync.dma_start(sel128_f[:], selbc)
            nc.vector.tensor_copy(sel128b[:], sel128_f[:])
            for q in range(4):
                nc.vector.memset(sel4_f[q][:], 0.0)
                nc.vector.memset(sel4_f[q][:, q:q + 1], 1.0)
                nc.vector.tensor_copy(sel4r[q][:], sel4_f[q][:])
            for h in range(HQ):
                t = wo_p.tile([128, 16 * HD], bf, tag="w", name=f"wqsb{h}")
                nc.sync.dma_start(t[:], wq[:, h * 2048:(h + 1) * 2048])
                wq_sb.append(t)

            def _qproj_finish(h, ps):
                grp, qh = divmod(h, 4)
                qraw = rtmp_p.tile([HD, QROWS], bf, tag="qraw", name=f"qraw{h}")
                nc.scalar.copy(qraw[:], ps[:])
                qt = qt_sb[grp][:, qh * QROWS:(qh + 1) * QROWS]
                t1 = rtmp_p.tile([HALF, QROWS], bf, tag="t1", name=f"qt1_{h}")
                t2 = rtmp_p.tile([HALF, QROWS], bf, tag="t2", name=f"qt2_{h}")
                nc.vector.tensor_mul(t1[:], qraw[0:HALF, :], cq[0:HALF, 0:QROWS])
                nc.vector.tensor_mul(t2[:], qraw[HALF:HD, :], sq[HALF:HD, 0:QROWS])
                nc.vector.tensor_sub(qt[0:HALF, :], t1[:], t2[:])
                t3 = rtmp_p.tile([HALF, QROWS], bf, tag="t1", name=f"qt3_{h}")
                t4 = rtmp_p.tile([HALF, QROWS], bf, tag="t2", name=f"qt4_{h}")
                nc.vector.tensor_mul(t3[:], qraw[0:HALF, :], sq[0:HALF, 0:QROWS])
                nc.vector.tensor_mul(t4[:], qraw[HALF:HD, :], cq[HALF:HD, 0:QROWS])
                nc.vector.tensor_add(qt[HALF:HD, :], t3[:], t4[:])

            def qproj_head(h):
                # one head -> one 512-col slice of qt_sb[h//4]
                ps = psQ_p.tile([HD, QROWS], f32, tag="qp", name=f"qps{h}")
                for kt in range(16):
                    nc.tensor.matmul(
                        ps[:],
                        wq_sb[h][:, kt * HD:(kt + 1) * HD],
                        xq_sb[:, kt * QROWS:(kt + 1) * QROWS],
                        start=(kt == 0), stop=(kt == 15))
                _qproj_finish(h, ps)

            # filler: qproj matmuls dribbled into attention-pass bubbles
            # (two per kv block, between the score and context matmuls)
            fillq = {"heads": [], "kt": 0, "ps": None}

            def fill_set(heads):
                fillq["heads"] = list(heads)

            def emit_fill(n):
                for _ in range(n):
                    if not fillq["heads"]:
                        return
                    h = fillq["heads"][0]
                    kt = fillq["kt"]
                    if kt == 0:
                        fillq["ps"] = psQ_p.tile([HD, QROWS], f32, tag="qp",
                                                 name=f"qps{h}")
                    nc.tensor.matmul(
                        fillq["ps"][:],
                        wq_sb[h][:, kt * HD:(kt + 1) * HD],
                        xq_sb[:, kt * QROWS:(kt + 1) * QROWS],
                        start=(kt == 0), stop=(kt == 15))
                    if kt == 15:
                        _qproj_finish(h, fillq["ps"])
                        fillq["heads"].pop(0)
                        fillq["kt"] = 0
                        fillq["ps"] = None
                    else:
                        fillq["kt"] = kt + 1

            def attn_unpack(kvh):
                base4 = 4 * kvh * PIECE
                for r in range(4):
                    rb = base4 + r * PIECE
                    nc.gpsimd.dma_start(
                        kt_sb[kvh][:, r * 512:(r + 1) * 512],
                        kv_out[rb:rb + 65536].rearrange("(p c) -> p c", p=HD))
                    nc.gpsimd.dma_start(
                        v_sb[kvh][:, r * 512:(r + 1) * 512].rearrange(
                            "p (b c) -> p b c", b=4),
                        kv_out[rb + 65536:rb + 2 * 65536].rearrange(
                            "(b p c) -> p b c", b=4, p=128))

            def attn_pass(kvh, pr):
                # one pass of 2 query heads over all kv blocks
                if True:
                    ctx = ctx_p.tile([HD, 2 * QROWS], f32, tag="ctx",
                                     name=f"ctx{kvh}_{pr}")
                    acc = a_p.tile([128, 2 * QROWS], f32r, tag="acc",
                                   name=f"acc{kvh}_{pr}")
                    for g in range(16):
                        off = 64 * (g // 2)
                        ng = QROWS - off
                        sc = ps2_p.tile([128, 2 * QROWS], f32, tag="p2",
                                        name=f"sc{kvh}_{g}_{pr}")
                        for hl in range(2):
                            qh = 2 * pr + hl
                            nc.tensor.matmul(
                                sc[:, hl * QROWS:hl * QROWS + ng],
                                kt_sb[kvh][:, g * 128:(g + 1) * 128],
                                qt_sb[kvh][:, qh * QROWS + off:(qh + 1) * QROWS],
                                start=True, stop=True)
                        emit_fill(2)
                        sc_src = sc.rearrange("p (q c) -> p q c", q=2)[:, :, 0:ng]
                        et = e_p.tile([128, 2 * QROWS], bf, tag="exp",
                                      name=f"et{kvh}_{g}_{pr}")
                        nc.scalar.activation(
                            et.rearrange("p (q c) -> p q c", q=2)[:, :, 0:ng],
                            sc_src,
                            Exp, scale=SCALE)
                        nc.vector.tensor_mul(
                            et.rearrange("p (q c) -> p q c", q=2)[:, :, 0:64],
                            et.rearrange("p (q c) -> p q c", q=2)[:, :, 0:64],
                            mask_sb.rearrange("p (g q c) -> p g q c", g=16, q=2)
                            [:, g, :, :])
                        with nc.allow_low_precision(reason="f32r softmax sums"):
                            if g == 0:
                                nc.vector.tensor_copy(acc[:], et[:])
                            else:
                                nc.vector.tensor_add(
                                    acc.rearrange("p (q c) -> p q c", q=2)
                                    [:, :, off:QROWS],
                                    acc.rearrange("p (q c) -> p q c", q=2)
                                    [:, :, off:QROWS],
                                    et.rearrange("p (q c) -> p q c", q=2)
                                    [:, :, 0:ng])
                        for hl in range(2):
                            nc.tensor.matmul(
                                ctx[:, hl * QROWS + off:(hl + 1) * QROWS],
                                v_sb[kvh][:, g * 128:(g + 1) * 128],
                                et[:, hl * QROWS:hl * QROWS + ng],
                                start=(g == 0), stop=(g == 15),
                                skip_group_check=True)
                    # tail: denominators -> reciprocal -> broadcast -> normalize
                    sums = ps2_p.tile([2, QROWS], f32, tag="p2",
                                      name=f"sums{kvh}_{pr}")
                    for hl in range(2):
                        nc.tensor.matmul(
                            sums[:], sel4r[hl][:, 0:2],
                            acc[:, hl * QROWS:(hl + 1) * QROWS],
                            start=(hl == 0), stop=(hl == 1),
                            skip_group_check=True)
                    recf = n_p.tile([2, QROWS], f32, tag="recf",
                                    name=f"recf{kvh}_{pr}")
                    nc.vector.reciprocal_approx_fast(recf[:], sums[:])
                    recb = n_p.tile([2, QROWS], bf, tag="recb",
                                    name=f"recb{kvh}_{pr}")
                    with nc.allow_low_precision(reason="bf16 softmax denominators"):
                        nc.vector.tensor_copy(recb[:], recf[:])
                    bps = ps2_p.tile([HD, 2 * QROWS], f32, tag="p2",
                                     name=f"bps{kvh}_{pr}")
                    for hl in range(2):
                        nc.tensor.matmul(
                            bps[:, hl * QROWS:(hl + 1) * QROWS],
                            sel128b[0:2, hl * HD:(hl + 1) * HD], recb[:],
                            start=True, stop=True)
                    bsb = b_p.tile([HD, 2 * QROWS], f32, tag="bsb",
                                   name=f"bsb{kvh}_{pr}")
                    nc.scalar.copy(bsb[:], bps[:])
                    nc.vector.tensor_mul(ctxn_sb[2 * kvh + pr][:], ctx[:], bsb[:])

            def wo_prefetch(kvh):
                for h in range(4 * kvh, 4 * kvh + 4):
                    t = wo_p.tile([HD, D], bf, tag="w", name=f"wosb{h}")
                    nc.sync.dma_start(t[:], wo[h * HD:(h + 1) * HD, :])
                    wo_sb[h] = t

            # 8 heads of lookahead before attention; heads 8-15 dribble
            # into the first four passes' bubbles two matmuls per kv block
            for h in range(8):
                qproj_head(h)
            attn_unpack(0)
            fill_set([8, 9])
            attn_pass(0, 0)
            fill_set([10, 11])
            attn_pass(0, 1)
            wo_prefetch(0)
            attn_unpack(1)
            fill_set([12, 13])
            attn_pass(1, 0)
            fill_set([14, 15])
            attn_pass(1, 1)
            wo_prefetch(1)
            attn_unpack(2)
            attn_pass(2, 0)
            attn_pass(2, 1)
            wo_prefetch(2)
            attn_unpack(3)
            attn_pass(3, 0)
            attn_pass(3, 1)
            wo_prefetch(3)

            for pool in [bpool, npool, apool, epool, psQ, ctx_ps, ps2,
                         rtmp, sbQ]:
                pool.__exit__(None, None, None)

            # ---------------- phase C: output projection ----------------
            with tc.tile_pool(name="out_ps", bufs=2, space="PSUM") as out_ps, \
                 tc.tile_pool(name="osb_pool", bufs=2) as osb_pool:
                for c in range(4):
                    ps = out_ps.tile([128, D], f32, tag="ops", name=f"ops{c}")
                    for sl in range(4):
                        for p in range(HQ // 2):
                            for hl in range(2):
                                h = 2 * p + hl
                                nc.tensor.matmul(
                                    ps[:, sl * 512:(sl + 1) * 512],
                                    ctxn_sb[p][:, hl * QROWS + c * 128:
                                               hl * QROWS + (c + 1) * 128],
                                    wo_sb[h][:, sl * 512:(sl + 1) * 512],
                                    start=(h == 0), stop=(h == HQ - 1))
                    osb = osb_pool.tile([128, D], f32, tag="osb", name=f"osb{c}")
                    nc.scalar.copy(osb[:], ps[:])
                    nc.sync.dma_start(out[c * 128:(c + 1) * 128, :], osb[:])
            wo_pool.__exit__(None, None, None)

    nc.compile()
    return nc


def _prep_core_inputs(c, Xq, Xkv, wq2, wk2, wv2, wo2, q_positions, kv_positions):
    bfl = ml_dtypes.bfloat16
    b, j = divmod(c, 4)
    chunks = CHUNKS[j]
    qrows = np.concatenate([np.arange(64 * ch, 64 * ch + 64) for ch in chunks])
    kvrows = np.arange(512 * j, 512 * j + 512)

    inv_freq = 1.0 / (MAX_TIMESCALE **
                      (2.0 * np.arange(HALF, dtype=np.float32) / HD))
    pq = q_positions[b][qrows].astype(np.float32)
    pk = kv_positions[b][kvrows].astype(np.float32)
    fq = inv_freq[:, None] * pq[None, :]
    fk = inv_freq[:, None] * pk[None, :]

    # validity mask for the first 64-col sub-block of each kv block:
    # chunk i0 = g//2, columns are rows 64*c0..64*c0+63, valid iff kv <= q
    dm = np.zeros((16, 128, 64), dtype=np.float32)
    for g in range(16):
        c0 = chunks[g // 2]
        kv_idx = 128 * g + np.arange(128)[:, None]
        q_idx = 64 * c0 + np.arange(64)[None, :]
        dm[g] = (kv_idx <= q_idx).astype(np.float32)
    dm = np.tile(dm[:, :, None, :], (1, 1, 4, 1)).reshape(16, 128, 256)

    cosq = np.concatenate([np.cos(fq)] * 2, axis=0)   # [128, 512]
    sinq = np.concatenate([np.sin(fq)] * 2, axis=0)
    coskv = np.concatenate([np.cos(fk)] * 2, axis=0)
    sinkv = np.concatenate([np.sin(fk)] * 2, axis=0)

    xq_dev = np.ascontiguousarray(
        Xq[b][qrows, :].T.reshape(16, 128, QROWS).transpose(1, 0, 2)
        .reshape(128, 16 * QROWS))
    xkv_dev = np.ascontiguousarray(
        Xkv[b][kvrows, :].T.reshape(16, 128, KVSH).transpose(1, 0, 2)
        .reshape(128, 16 * KVSH))
    return dict(
        xqT=xq_dev.astype(bfl),
        xkvT=xkv_dev.astype(bfl),
        wq=wq2, wk=wk2, wv=wv2, wo=wo2,
        cosq2=np.tile(cosq, (1, 2)).astype(bfl),
        sinq2=np.tile(sinq, (1, 2)).astype(bfl),
        coskv4=np.tile(coskv, (1, 4)).astype(bfl),
        sinkv4=np.tile(sinkv, (1, 4)).astype(bfl),
        dmask=dm.astype(bfl),
        selbc=_selbc(),
    )


def _selbc():
    s = np.zeros((4, 4 * HD), dtype=np.float32)
    for q in range(4):
        s[q, q * HD:(q + 1) * HD] = 1.0
    return s


def kernel(Xq, Xkv, Wq, Wk, Wv, Wo, q_positions, kv_positions):
    from concourse import bass_utils

    Xq = np.asarray(Xq, dtype=np.float32)
    Xkv = np.asarray(Xkv, dtype=np.float32)
    Wq = np.asarray(Wq, dtype=np.float32)
    Wk = np.asarray(Wk, dtype=np.float32)
    Wv = np.asarray(Wv, dtype=np.float32)
    Wo = np.asarray(Wo, dtype=np.float32)
    q_positions = np.asarray(q_positions)
    kv_positions = np.asarray(kv_positions)

    if "nc" not in _CACHE:
        _CACHE["nc"] = _build()
    nc = _CACHE["nc"]

    bfl = ml_dtypes.bfloat16
    # wq: [128 p, h*16kt*128] so each head's lhsT block is contiguous
    wq2 = np.ascontiguousarray(
        Wq.reshape(16, 128, HQ, HD).transpose(1, 2, 0, 3)
        .reshape(128, HQ * 16 * HD)).astype(bfl)
    wk2 = np.ascontiguousarray(
        Wk.reshape(16, 128, HKV * HD).transpose(1, 0, 2)
        .reshape(128, 16 * HKV * HD)).astype(bfl)
    wv2 = np.ascontiguousarray(
        Wv.reshape(16, 128, HKV * HD).transpose(1, 0, 2)
        .reshape(128, 16 * HKV * HD)).astype(bfl)
    wo2 = np.ascontiguousarray(Wo.reshape(HQ * HD, D)).astype(bfl)

    in_maps = [_prep_core_inputs(c, Xq, Xkv, wq2, wk2, wv2, wo2,
                                 q_positions, kv_positions)
               for c in range(N_CORES)]

    res = bass_utils.run_bass_kernel_spmd(
        nc, in_maps, core_ids=list(range(N_CORES)),
        **_CACHE.get("run_kwargs", {}))
    _CACHE["last_results"] = res

    out = np.empty((B, T, D), dtype=np.float32)
    for c in range(N_CORES):
        b, j = divmod(c, 4)
        core_out = res.results[c]["out"]
        for i, ch in enumerate(CHUNKS[j]):
            out[b, 64 * ch:64 * ch + 64, :] = core_out[64 * i:64 * i + 64, :]
    return out

